# revision 27
# baseline (speedup 1.0000x reference)
import sys

sys.path.insert(0, "/opt/trn_rl_repo")

import numpy as np
import ml_dtypes

import concourse.bass as bass
import concourse.bacc as bacc
import concourse.tile as tile
from concourse.bass_utils import run_bass_kernel_spmd
from concourse import mybir

B, L, D, H = 2, 2048, 1024, 16
DH = 64          # dim per head
HPC = 4          # heads per core
CPC = HPC * DH   # feature cols per core = 256
NCORES = 8

MM_DT = "bfloat16"
NP_MM = ml_dtypes.bfloat16 if MM_DT == "bfloat16" else np.float32

_CACHE = {}


def build_nc(mm_dt: str):
    nc = bacc.Bacc()
    mm_dt = mybir.dt(mm_dt)
    fp32 = mybir.dt.float32

    # x layouts host-prepped to [128, 4(ch), 8(dc), 512] so each DMA chunk is
    # contiguous per partition (16KB lines)
    xq = nc.declare_dram_parameter("xq", (128, 4, 8, 512), mm_dt, isOutput=False)
    xk = nc.declare_dram_parameter("xk", (128, 4, 8, 512), mm_dt, isOutput=False)
    xv = nc.declare_dram_parameter("xv", (128, 4, 8, 512), mm_dt, isOutput=False)
    wq = nc.declare_dram_parameter("wq", (128, 8, CPC), mm_dt, isOutput=False)
    wk = nc.declare_dram_parameter("wk", (128, 8, CPC), mm_dt, isOutput=False)
    wv = nc.declare_dram_parameter("wv", (128, 8, CPC), mm_dt, isOutput=False)
    wo = nc.declare_dram_parameter("wo", (2, 128, D), mm_dt, isOutput=False)
    bq = nc.declare_dram_parameter("bq", (128, 2), fp32, isOutput=False)
    bk = nc.declare_dram_parameter("bk", (128, 2), fp32, isOutput=False)
    y = nc.declare_dram_parameter("y", (L, D), mm_dt, isOutput=True)  # partial out

    from contextlib import ExitStack

    with ExitStack() as es:
        tc = es.enter_context(tile.TileContext(nc))
        # NOTE: bufs are per named tag
        xt_pool = es.enter_context(tc.tile_pool(name="xt", bufs=1))     # 3 tags [128,4,8,512]
        w_pool = es.enter_context(tc.tile_pool(name="w", bufs=1))       # 3 tags [128,8,256]
        wo_pool = es.enter_context(tc.tile_pool(name="wo", bufs=1))     # 2 tags [128,1024]
        bias_pool = es.enter_context(tc.tile_pool(name="bias", bufs=1))
        qt_pool = es.enter_context(tc.tile_pool(name="qt", bufs=1))     # 2 tags [128,2048]
        kt_pool = es.enter_context(tc.tile_pool(name="kt", bufs=1))
        vn_pool = es.enter_context(tc.tile_pool(name="vn", bufs=1))     # [128,16,4,65]
        pt_pool = es.enter_context(tc.tile_pool(name="pt", bufs=6))     # [128,1024]
        zr_pool = es.enter_context(tc.tile_pool(name="zr", bufs=3))     # [1,512]
        zbs_pool = es.enter_context(tc.tile_pool(name="zbs", bufs=3))   # [64,512]
        ot_pool = es.enter_context(tc.tile_pool(name="ot", bufs=1))     # 2 tags [128,2048]
        y_pool = es.enter_context(tc.tile_pool(name="ysb", bufs=4))     # [128,512]
        psA = es.enter_context(tc.tile_pool(name="psA", bufs=2, space="PSUM"))
        psS = es.enter_context(tc.tile_pool(name="psS", bufs=2, space="PSUM"))
        psOT = es.enter_context(tc.tile_pool(name="psOT", bufs=2, space="PSUM"))
        if True:
            # ---- load inputs (DMA queue order == consumption order) ---------
            wk_sb = w_pool.tile([128, 8, CPC], mm_dt, name="wk")
            nc.sync.dma_start(out=wk_sb[:, 0:2], in_=wk[:, 0:2])
            nc.sync.dma_start(out=wk_sb[:, 2:8], in_=wk[:, 2:8])
            bk_sb = bias_pool.tile([128, 2], fp32, name="bk")
            nc.sync.dma_start(out=bk_sb, in_=bk[:, 0:2])
            xk_sb = xt_pool.tile([128, 4, 8, L // 4], mm_dt, name="xk")
            nc.sync.dma_start(out=xk_sb[:, 0, 0:2], in_=xk[:, 0, 0:2])
            nc.sync.dma_start(out=xk_sb[:, 0, 2:4], in_=xk[:, 0, 2:4])
            nc.sync.dma_start(out=xk_sb[:, 0, 4:8], in_=xk[:, 0, 4:8])

            wq_sb = w_pool.tile([128, 8, CPC], mm_dt, name="wq")
            nc.sync.dma_start(out=wq_sb, in_=wq[:, 0:8])
            bq_sb = bias_pool.tile([128, 2], fp32, name="bq")
            nc.sync.dma_start(out=bq_sb, in_=bq[:, 0:2])
            xq_sb = xt_pool.tile([128, 4, 8, L // 4], mm_dt, name="xq")
            nc.sync.dma_start(out=xq_sb[:, 0, 0:4], in_=xq[:, 0, 0:4])
            nc.sync.dma_start(out=xq_sb[:, 0, 4:8], in_=xq[:, 0, 4:8])

            wv_sb = w_pool.tile([128, 8, CPC], mm_dt, name="wv")
            nc.sync.dma_start(out=wv_sb, in_=wv[:, 0:8])
            xv_sb = xt_pool.tile([128, 4, 8, L // 4], mm_dt, name="xv")
            nc.sync.dma_start(out=xv_sb[:, 0, 0:4], in_=xv[:, 0, 0:4])
            nc.sync.dma_start(out=xv_sb[:, 0, 4:8], in_=xv[:, 0, 4:8])

            wo_sb = [wo_pool.tile([128, D], mm_dt, name=f"wo{cc}")
                     for cc in range(2)]
            for ch in range(1, 4):
                if ch == 3:
                    # wo is needed for C(0), before the ch3 fillers consume
                    nc.sync.dma_start(out=wo_sb[0], in_=wo[0])
                    nc.sync.dma_start(out=wo_sb[1], in_=wo[1])
                nc.sync.dma_start(out=xk_sb[:, ch], in_=xk[:, ch])
                nc.sync.dma_start(out=xq_sb[:, ch], in_=xq[:, ch])
                nc.sync.dma_start(out=xv_sb[:, ch], in_=xv[:, ch])

            # ---- stage A helpers (emitted chunk-wise, interleaved with B) ---
            qt_sb = [qt_pool.tile([128, L], mm_dt, name=f"qt{i}") for i in range(2)]
            kt_sb = [kt_pool.tile([128, L], mm_dt, name=f"kt{i}") for i in range(2)]
            # V natural layout: [128(lt-part), 16 lt, 4 head, 65] (col 64 = ones)
            v_sb = vn_pool.tile([128, 16, 4, 65], mm_dt)
            nc.vector.memset(v_sb[:, :, :, 64:65], 1.0)
            # lower-triangle keep-mask for diag strips: mask[p,f] = (f >= p);
            # built once so the per-strip zeroing is a cheap DVE multiply
            # instead of a gpsimd affine_select
            tri_sb = bias_pool.tile([128, 128], mm_dt, name="tri")
            nc.vector.memset(tri_sb, 1.0)
            nc.gpsimd.affine_select(
                out=tri_sb, in_=tri_sb,
                compare_op=mybir.AluOpType.is_ge,
                fill=0.0, base=0, channel_multiplier=-1,
                pattern=[[1, 128]],
            )
            # dummy exp: pull the ACT table load under the initial DMA wait
            # instead of the first S group's critical path
            warm_sb = bias_pool.tile([1, 1], fp32, name="warm")
            nc.scalar.activation(
                out=warm_sb, in_=tri_sb[0:1, 0:1],
                func=mybir.ActivationFunctionType.Exp,
                scale=0.125,
            )

            def emit_QK(dst, x_sb, w_sb, b_sb, lg):
                for cc in range(2):
                    ps = psA.tile([128, 512], fp32)
                    for dc in range(8):
                        nc.tensor.matmul(
                            ps,
                            w_sb[:, dc, cc * 128:(cc + 1) * 128],
                            x_sb[:, lg, dc, :],
                            start=(dc == 0),
                            stop=(dc == 7),
                        )
                    nc.vector.tensor_scalar_add(
                        out=dst[cc][:, lg * 512:(lg + 1) * 512],
                        in0=ps,
                        scalar1=b_sb[:, cc:cc + 1],
                    )

            def emit_V(lt):
                ps = psA.tile([128, CPC], fp32)
                for dc in range(8):
                    nc.tensor.matmul(
                        ps,
                        xv_sb[:, lt // 4, dc, (lt % 4) * 128:(lt % 4 + 1) * 128],
                        wv_sb[:, dc, :],
                        start=(dc == 0),
                        stop=(dc == 7),
                    )
                nc.vector.tensor_copy(
                    out=v_sb[:, lt, :, 0:64],
                    in_=ps.rearrange("p (h d) -> p h d", d=64),
                )

            # prologue: just enough of A to start B(g4=0)
            emit_QK(kt_sb, xk_sb, wk_sb, bk_sb, 0)
            emit_QK(qt_sb, xq_sb, wq_sb, bq_sb, 0)
            for lt in range(4):
                emit_V(lt)

            # ---- stage B + C interleaved ------------------------------------
            ot_sb = [ot_pool.tile([128, L], mm_dt, name=f"ot{i}") for i in range(2)]
            y_view = y.rearrange("(lt p) c -> p lt c", p=128)

            def emit_C(g4, dve_only=False, lts=(0, 1, 2, 3)):
                # dve_only for mid-kernel C groups: ACT is saturated with exps
                # there, DVE has slack
                for li in lts:
                    lt = g4 * 4 + li
                    for dg in range(2):
                        ps = psA.tile([128, 512], fp32)
                        for cc in range(2):
                            nc.tensor.matmul(
                                ps,
                                ot_sb[cc][:, lt * 128:(lt + 1) * 128],
                                wo_sb[cc][:, dg * 512:(dg + 1) * 512],
                                start=(cc == 0),
                                stop=(cc == 1),
                            )
                        yt = y_pool.tile([128, 512], mm_dt)
                        if dve_only or dg == 0:
                            nc.vector.tensor_copy(out=yt, in_=ps)
                        else:
                            nc.scalar.activation(
                                out=yt, in_=ps,
                                func=mybir.ActivationFunctionType.Copy,
                                bias=0.0,
                            )
                        nc.sync.dma_start(
                            out=y_view[:, lt, dg * 512:(dg + 1) * 512],
                            in_=yt,
                        )

            for g4 in range(4):
                for h in range(HPC):
                    cc = h // 2
                    ro = (h % 2) * 64
                    nkt = g4 * 4 + 4
                    ot_ps = psOT.tile([65, 512], fp32)
                    pts = {}

                    def emit_S_pair(k0):
                        # two kt tiles share a [128,1024] PSUM pair; non-diag
                        # pairs get a single wide exp (saves ACT overhead)
                        diag = (k0 // 4 == g4)
                        st = psS.tile([128, 1024], fp32, name="st2")
                        for j in range(2):
                            kt = k0 + j
                            off = 128 * (kt % 4) if diag else 0
                            base = j * 512
                            nc.tensor.matmul(
                                st[:, base + off:base + 512],
                                kt_sb[cc][ro:ro + 64, kt * 128:(kt + 1) * 128],
                                qt_sb[cc][ro:ro + 64,
                                          g4 * 512 + off:(g4 + 1) * 512],
                                start=True,
                                stop=True,
                            )
                        pt = pt_pool.tile([128, 1024], mm_dt, name="pt2")
                        if not diag:
                            nc.scalar.activation(
                                out=pt,
                                in_=st,
                                func=mybir.ActivationFunctionType.Exp,
                                scale=0.125,
                            )
                        else:
                            # one strided exp covers both tiles from off0; the
                            # 128 extra cols of tile j=1 are garbage but sit
                            # below its diagonal, which PV skips entirely
                            off0 = 128 * (k0 % 4)
                            st_v = st.rearrange("p (j c) -> p j c", j=2)
                            pt_v = pt.rearrange("p (j c) -> p j c", j=2)
                            nc.scalar.activation(
                                out=pt_v[:, :, off0:512],
                                in_=st_v[:, :, off0:512],
                                func=mybir.ActivationFunctionType.Exp,
                                scale=0.125,
                            )
                            for j in range(2):
                                off = off0 + 128 * j
                                base = j * 512
                                # zero the below-diagonal cols of the boundary
                                # 128-strip; cols < off are skipped by PV
                                nc.vector.tensor_mul(
                                    out=pt[:, base + off:base + off + 128],
                                    in0=pt[:, base + off:base + off + 128],
                                    in1=tri_sb,
                                )
                        pts[k0] = pt[:, 0:512]
                        pts[k0 + 1] = pt[:, 512:1024]

                    last = (g4 == 3 and h == 3)

                    def emit_P(kt):
                        # diag tiles: queries < off are fully masked, skip them
                        diag = (kt // 4 == g4)
                        off = 128 * (kt % 4) if diag else 0
                        # on the very last head, mark each diag tile done so
                        # the divide can start per 128-col quarter (no later
                        # PV touches cols < off+128; stop is sim-bookkeeping)
                        stop = (kt == nkt - 1) or (last and diag)
                        nc.tensor.matmul(
                            ot_ps[:, off:512],
                            v_sb[:, kt, h, :],
                            pts.pop(kt)[:, off:512],
                            start=(kt == 0),
                            stop=stop,
                            skip_group_check=True,
                        )

                    def emit_div_half(hi):
                        # divide half hi (256 cols) of the last head's ot as
                        # soon as those columns' Z is final
                        c0 = hi * 256
                        zrow = zr_pool.tile([1, 256], fp32, name="zrowh")
                        nc.vector.tensor_copy(
                            out=zrow, in_=ot_ps[64:65, c0:c0 + 256])
                        zh = zr_pool.tile([1, 256], fp32, name="zrh")
                        nc.vector.reciprocal_approx_fast(out=zh, in_=zrow)
                        zb = zbs_pool.tile([64, 256], fp32, name="zbh")
                        nc.gpsimd.partition_broadcast(out_ap=zb, in_ap=zh)
                        nc.vector.tensor_mul(
                            out=ot_sb[cc][ro:ro + 64,
                                          g4 * 512 + c0:g4 * 512 + c0 + 256],
                            in0=ot_ps[0:64, c0:c0 + 256],
                            in1=zb,
                        )

                    npair = nkt // 2
                    for kp in range(npair):
                        emit_S_pair(2 * kp)
                        if kp >= 1:
                            emit_P(2 * kp - 2)
                            emit_P(2 * kp - 1)
                    if last:
                        # PV(13) closed cols 0:256; divide that half while
                        # PV(14,15) and the last C(2) piece keep the PE fed,
                        # then the rest, then final C per lt
                        emit_div_half(0)
                        emit_P(nkt - 2)
                        emit_C(2, dve_only=True, lts=(3,))
                        emit_P(nkt - 1)
                        emit_div_half(1)
                        for li in range(4):
                            emit_C(3, lts=(li,))
                    else:
                        emit_P(nkt - 2)
                        emit_P(nkt - 1)

                    # divide by Z (row 64) -- off the PE entirely.
                    # NB: reciprocal_approx_fast reading PSUM directly is
                    # silently wrong; bounce the row through SBUF first.
                    # (the very last head divides inline, per quarter, above)
                    if not last:
                        zrow = zr_pool.tile([1, 512], fp32, name="zrow")
                        nc.vector.tensor_copy(out=zrow, in_=ot_ps[64:65, :])
                        zr = zr_pool.tile([1, 512], fp32, name="zr")
                        nc.vector.reciprocal_approx_fast(out=zr, in_=zrow)
                        zb = zbs_pool.tile([64, 512], fp32)
                        nc.gpsimd.partition_broadcast(out_ap=zb, in_ap=zr)
                        nc.vector.tensor_mul(
                            out=ot_sb[cc][ro:ro + 64, g4 * 512:(g4 + 1) * 512],
                            in0=ot_ps[0:64, :],
                            in1=zb,
                        )
                    # fillers: projection chunks for g4+1 keep the PE busy
                    # while this group's exp/affine/divide chains drain.
                    # g4=3 has no A-stage left; spread the remaining C there.
                    if g4 < 3:
                        nx = g4 + 1
                        if h == 0:
                            emit_QK(kt_sb, xk_sb, wk_sb, bk_sb, nx)
                        elif h == 1:
                            emit_QK(qt_sb, xq_sb, wq_sb, bq_sb, nx)
                        elif h == 2:
                            emit_V(4 * nx)
                            emit_V(4 * nx + 1)
                        else:
                            emit_V(4 * nx + 2)
                            emit_V(4 * nx + 3)
                        if h == 2 and g4 > 0:
                            emit_C(g4 - 1)
                    else:
                        if h == 0:
                            emit_C(2, dve_only=True, lts=(0, 1))
                        elif h == 1:
                            emit_C(2, dve_only=True, lts=(2,))

    nc.compile()
    return nc


def _get_nc(mm_dt: str):
    if mm_dt not in _CACHE:
        _CACHE[mm_dt] = build_nc(mm_dt)
    return _CACHE[mm_dt]


def _prep_x(x):
    # [L, D] -> [128(p), 4(ch), 8(dc), 512] with x_prep[p,ch,dc,cw] = x[ch*512+cw, dc*128+p]
    return np.ascontiguousarray(
        x.reshape(4, 512, 8, 128).transpose(3, 0, 2, 1)
    ).astype(NP_MM)


def _prep_w(w):
    # [D, CPC] -> [128(p), 8(dc), CPC]
    return np.ascontiguousarray(
        w.reshape(8, 128, CPC).transpose(1, 0, 2)
    ).astype(NP_MM)


def kernel(q, k, v, mask, Wq, bq, Wk, bk, Wv, bv, Wo, bo, _trace=False):
    nc = _get_nc(MM_DT)

    in_maps = []
    for c in range(NCORES):
        b = c // 4
        g = c % 4
        s = slice(g * CPC, (g + 1) * CPC)
        in_maps.append({
            "xq": _prep_x(np.asarray(q[b])),
            "xk": _prep_x(np.asarray(k[b])),
            "xv": _prep_x(np.asarray(v[b])),
            "wq": _prep_w(np.asarray(Wq[:, s])),
            "wk": _prep_w(np.asarray(Wk[:, s])),
            "wv": _prep_w(np.asarray(Wv[:, s])),
            "wo": np.ascontiguousarray(Wo[s, :].reshape(2, 128, D)).astype(NP_MM),
            "bq": np.ascontiguousarray(bq[s].reshape(2, 128).T).astype(np.float32),
            "bk": np.ascontiguousarray(bk[s].reshape(2, 128).T).astype(np.float32),
        })

    res = run_bass_kernel_spmd(nc, in_maps, list(range(NCORES)), trace=_trace)

    # host gather: out[b] = sum_g y_core(b,g) + (bo + bv @ Wo)
    const = (bo + bv.astype(np.float64) @ Wo.astype(np.float64)).astype(np.float64)
    out = np.zeros((B, L, D), np.float64)
    for c in range(NCORES):
        out[c // 4] += res.results[c]["y"].astype(np.float64)
    out += const[None, None, :]
    kernel.last_exec_time_ns = res.exec_time_ns
    return out.astype(np.float32)
